# revision 31
# baseline (speedup 1.0000x reference)
"""Trainium2 Bass kernel for windowless relative-position-bias attention.

Problem (hardcoded shapes):
  x [16, 1024, 512] f32, W_qkv [512, 1536], rel_table [3969, 8],
  W_out [512, 512], b_out [512], rel_index [1048576] i32 (canonical
  32x32 relative-position pattern; only its structure is used).

Sharding: tensor-parallel over heads -- core c owns head c for all 16
batches; the final projection is data-parallel over batches (core c
produces output batches 2c, 2c+1) after an on-chip AllToAll of the
per-head attention outputs.

Per core (head h = core id) the 16 batches run as a 3-deep software
pipeline: iteration i emits batch b_i's softmax+PV, batch b_{i+1}'s
projections (qT/kT/v) interleaved into the PE stream, and batch
b_{i-1}'s normalization -- so TensorE never idles at batch boundaries
and stays at the warm 2.4 GHz HAM clock.

Softmax (max-subtraction skipped; logits bounded ~|9|) splits the
exp+bias work per mc-block across three engines:
  * a-blocks: ScalarE exp(SCALE*dots), then multiply with the
    host-precomputed exp(bias) on VectorE or GpSimd.
  * d-blocks: fused Schraudolph exp on VectorE -- one custom DVE op
    i16 = (dots*SCALE*A + C1) + A*bias, bits reinterpreted as fp16
    (A = 1024*log2(e); C1 centers the piecewise-linear exp2 error so
    softmax ratios stay unbiased; adds ~0.8% rel err, budget is 2%).
The PV matmul gets the softmax denominator for free via a ones-column
appended to v; normalization is a K=1 reciprocal-broadcast matmul.

AllToAll halves: even batches exchange while odd batches compute; the
half-A output projection is interleaved into late iterations, only
half-B remains in the tail.  A tiny warm-up AllToAll doubles as a
cross-core start-skew barrier: batch 0's PV consumes one value from it.
"""

import os
import sys

for _p in ("/opt/trn_rl_repo", "/root/.axon_site/_ro/trn_rl_repo"):
    if os.path.isdir(_p) and _p not in sys.path:
        sys.path.insert(0, _p)

import numpy as np
import ml_dtypes

import concourse.bass as bass
import concourse.mybir as mybir
import concourse.tile as tile
from concourse import bacc
from concourse.bass import AP
from concourse.bass_utils import run_bass_kernel_spmd

# Content-hash NEFF cache: identical BIR -> reuse the compiled NEFF
# (neuronxcc is ~6 min; this makes repeat runs seconds).
import concourse.bass_utils as _bu
import concourse.bass2jax as _b2j

_orig_compile_bir = _bu.compile_bir_kernel


def _cached_compile_bir(bir_json, tmpdir, neff_name="file.neff"):
    import hashlib
    import shutil
    h = hashlib.sha256(bir_json).hexdigest()[:24]
    cdir = os.environ.get("NEFF_CACHE_DIR", "/tmp/neff_cache")
    os.makedirs(cdir, exist_ok=True)
    cpath = os.path.join(cdir, h + ".neff")
    if os.path.exists(cpath):
        dst = os.path.join(tmpdir, neff_name)
        shutil.copy(cpath, dst)
        return dst
    p = _orig_compile_bir(bir_json, tmpdir, neff_name)
    try:
        shutil.copy(p, cpath)
    except OSError:
        pass
    return p


_bu.compile_bir_kernel = _cached_compile_bir
_b2j.compile_bir_kernel = _cached_compile_bir

B, IH, IW = 16, 32, 32
N = IH * IW          # 1024
H, D = 8, 64
INNER = H * D        # 512
INP = OUP = 512
SCALE = D ** -0.5    # 0.125
NCORES = 8
BPC = B // NCORES    # batches per core = 2
TBL = (2 * IH - 1) * (2 * IW - 1)  # 3969

# Schraudolph fp16 exp constants
A_EXP = 1024.0 * np.log2(np.e)          # 1477.3197
C1_EXP = 15360.0 - 59.0 + 0.5           # exponent offset - mean-center - trunc

# per-batch mc-block engine assignment (tunable):
A_DVE = (0,)          # ScalarE exp + VectorE bias-mult
A_GPS = (1, 2)        # ScalarE exp + GpSimd bias-mult
D_SCH = (3, 4, 5, 6, 7)   # VectorE fused Schraudolph (incl. bias)
A_BLK = A_DVE + A_GPS

F32 = mybir.dt.float32
F32R = mybir.dt.float32r
F16 = mybir.dt.float16
BF16 = mybir.dt.bfloat16
I16 = mybir.dt.int16


def build_nc():
    nc = bacc.Bacc("TRN2", target_bir_lowering=False, num_devices=NCORES)

    xt_d = nc.dram_tensor("xt", [INP, B * N], F16, kind="ExternalInput")
    wqk_d = nc.dram_tensor("wqk", [INP, 128], F16, kind="ExternalInput")
    wv_d = nc.dram_tensor("wv", [INP, D], F16, kind="ExternalInput")
    texp_d = nc.dram_tensor("texp", [len(A_BLK) * 128, N], F16, kind="ExternalInput")
    tba_d = nc.dram_tensor("tba", [len(D_SCH) * 128, N], F16, kind="ExternalInput")
    wout_d = nc.dram_tensor("wout", [INNER, OUP], F16, kind="ExternalInput")
    bout_d = nc.dram_tensor("bout", [1, OUP], BF16, kind="ExternalInput")
    ones_d = nc.dram_tensor("ones", [1024], BF16, kind="ExternalInput")
    out_d = nc.dram_tensor("out", [BPC * N, OUP], F32, kind="ExternalOutput")

    with tile.TileContext(nc) as tc:
        with (
            tc.tile_pool(name="consts", bufs=1) as consts,
            tc.tile_pool(name="xt", bufs=3) as xtp,
            tc.tile_pool(name="qkt", bufs=2) as qktp,
            tc.tile_pool(name="ktq", bufs=2) as ktqp,
            tc.tile_pool(name="vaug", bufs=2) as vaugp,
            tc.tile_pool(name="attn", bufs=4) as attnp,
            tc.tile_pool(name="atti", bufs=3) as attip,
            tc.tile_pool(name="small", bufs=2) as smallp,
            tc.tile_pool(name="o2b", bufs=2) as o2bp,
            tc.tile_pool(name="lh", bufs=1) as lhp,
            tc.tile_pool(name="outp", bufs=2) as outp,
            tc.tile_pool(name="pswork", bufs=3, space="PSUM") as pswork,
            tc.tile_pool(name="psacc", bufs=1, space="PSUM") as psacc,
            tc.tile_pool(name="dram", bufs=1, space="DRAM") as dramp,
        ):
            batch_order = list(range(0, B, 2)) + list(range(1, B, 2))

            # ---- x^T for batches 0,1 queued before anything slow ----
            def load_xt(b):
                xt = xtp.tile([128, 4, N], F16)
                for ic in range(4):
                    eng = nc.sync if ic < 2 else nc.gpsimd
                    eng.dma_start(
                        out=xt[:, ic, :],
                        in_=xt_d[ic * 128:(ic + 1) * 128, b * N:(b + 1) * N])
                return xt

            xt_tiles = {0: load_xt(batch_order[0]),
                        1: load_xt(batch_order[1])}

            # ---- collectives warm-up; also the cross-core skew barrier
            # (batch 0's PV consumes one exchanged value) ----
            cc_inA = dramp.tile([NCORES, D, N], F16, tag="ccinA")
            cc_outA = dramp.tile([NCORES, D, N], F16, tag="ccoutA")
            cc_inB = dramp.tile([NCORES, D, N], F16, tag="ccinB")
            cc_outB = dramp.tile([NCORES, D, N], F16, tag="ccoutB")
# no warm-up collective: its peer-skew wait serializes DMA-ring traffic
            # behind it for ~25us at startup; A2A-A at i=8 absorbs firmware
            # init asynchronously (3 iterations of slack before lhA is read)

            # ---- weights ----
            wqk_sb = consts.tile([128, 4, 128], F16, tag="wqk")
            wv_sb = consts.tile([128, 4, D], F16, tag="wv")
            wout_sb = consts.tile([128, 4, OUP], F16, tag="wout")
            bout_sb = consts.tile([65, OUP], BF16, tag="bout")
            ones1 = consts.tile([65, 128], BF16, tag="ones")
            ones0 = consts.tile([1, 128], BF16, tag="ones0")
            for ic in range(4):
                nc.sync.dma_start(out=wqk_sb[:, ic, :], in_=wqk_d[ic * 128:(ic + 1) * 128, :])
                nc.sync.dma_start(out=wv_sb[:, ic, :], in_=wv_d[ic * 128:(ic + 1) * 128, :])
                nc.gpsimd.dma_start(out=wout_sb[:, ic, :], in_=wout_d[ic * 128:(ic + 1) * 128, :])
            nc.sync.dma_start(out=bout_sb[64:65, :], in_=bout_d[:])
            nc.sync.dma_start(out=ones1[64:65, :], in_=ones_d[0:128])
            nc.sync.dma_start(out=ones0[:], in_=ones_d[0:128])

            # ---- bias tables (host-expanded per-(m,n); contiguous DMA) ----
            expb = consts.tile([128, len(A_BLK), N], F16, tag="expb")
            biasA = consts.tile([128, len(D_SCH), N], F16, tag="biasA")
            for slot in range(len(A_BLK)):
                nc.sync.dma_start(out=expb[:, slot, :],
                                  in_=texp_d[slot * 128:(slot + 1) * 128, :])
            for slot in range(len(D_SCH)):
                nc.gpsimd.dma_start(out=biasA[:, slot, :],
                                    in_=tba_d[slot * 128:(slot + 1) * 128, :])

            lhA = lhp.tile([128, 4, N], F16, tag="lhA")
            lhB = lhp.tile([128, 4, N], F16, tag="lhB")

            def out_chunk(nq, lh):
                """output-projection chunk: out rows nq*128..+128."""
                ps_f = pswork.tile([128, OUP], F32, tag="big")
                for kc in range(4):
                    nc.tensor.matmul(
                        ps_f[:],
                        lh[:, kc, (nq % 8) * 128:(nq % 8 + 1) * 128],
                        wout_sb[:, kc, :],
                        start=(kc == 0), stop=False)
                nc.tensor.matmul(ps_f[:], ones1[64:65, :], bout_sb[64:65, :],
                                 start=False, stop=True)
                o_sb = outp.tile([128, OUP], F32)
                nc.scalar.copy(o_sb[:], ps_f[:])
                nc.sync.dma_start(out=out_d[nq * 128:(nq + 1) * 128, :],
                                  in_=o_sb[:])

            def normalize(state):
                """1/denominator broadcast + multiply + ship to cc buffer.

                The K=1 broadcast matmul runs in bf16 (an f32/f32r one
                lowers to the 4-cycle/row two-pass fp32 path, ~1.4us of
                TensorE per batch; bf16 is full rate and the denominator
                only needs ~3 digits).
                """
                o_ps, dn, b = state
                rb_ps = pswork.tile([D, N], F32, tag="big")
                for fc in range(2):
                    nc.tensor.matmul(
                        rb_ps[:, fc * 512:(fc + 1) * 512],
                        ones0[0:1, 0:D],
                        dn[0:1, fc * 512:(fc + 1) * 512],
                        start=True, stop=True)
                rb = smallp.tile([D, N], F32, tag="rb")
                nc.vector.reciprocal_approx_fast(rb[:], rb_ps[:])
                o2b = o2bp.tile([D, N], F16)
                nc.vector.tensor_tensor(o2b[:], o_ps[0:D, :], rb[:],
                                        mybir.AluOpType.mult)
                cc_dst = cc_inA if b % 2 == 0 else cc_inB
                nc.sync.dma_start(out=cc_dst[b // BPC], in_=o2b[:])

            def project(b, xt, barrier=False):
                """qT/kT + v for batch b; returns (qkt, ktq_lo_pending, vaug).

                Emits the PE matmuls + casts + the sync-queue k copy; the
                gpsimd-queue q duplicate is deferred (emit_ktq_hi) so it
                lands after this iteration's gpsimd multiplies.
                """
                qkt_ps = pswork.tile([128, N], F32, tag="big")
                for fc in range(2):
                    for ic in range(4):
                        nc.tensor.matmul(
                            qkt_ps[:, fc * 512:(fc + 1) * 512],
                            wqk_sb[:, ic, :],
                            xt[:, ic, fc * 512:(fc + 1) * 512],
                            start=(ic == 0), stop=(ic == 3))
                qkt = qktp.tile([128, N], F16)
                nc.scalar.copy(qkt[:], qkt_ps[:])
                ktq = ktqp.tile([128, N], F16)
                nc.sync.dma_start(out=ktq[0:64, :], in_=qkt[64:128, :])
                return qkt, ktq

            def project_v(b, xt):
                vaug = vaugp.tile([128, 8, D + 1], F16)
                nc.vector.memset(vaug[:, :, D], 1.0)
                v_ps = pswork.tile([128, 8, D], F32, tag="big")
                for nc_ in range(8):
                    for ic in range(4):
                        nc.tensor.matmul(
                            v_ps[:, nc_, :],
                            xt[:, ic, nc_ * 128:(nc_ + 1) * 128],
                            wv_sb[:, ic, :],
                            start=(ic == 0), stop=(ic == 3))
                nc.scalar.copy(vaug[:, :, 0:D], v_ps[:])
                return vaug

            # ---- prologue: batch 0 projections ----
            qkt_cur, ktq_cur = project(batch_order[0], xt_tiles[0])
            nc.gpsimd.dma_start(out=ktq_cur[64:128, :], in_=qkt_cur[0:64, :])
            vaug_cur = project_v(batch_order[0], xt_tiles[0])

            pend = None  # deferred normalize state

            for i, b in enumerate(batch_order):
                xt = xt_tiles.pop(i)
                last = i == B - 1
                nxt = batch_order[i + 1] if not last else None
                qkt, ktq, vaug = qkt_cur, ktq_cur, vaug_cur

                # --- normalization of the previous batch (PE: 2 tiny MMs) ---
                if pend is not None:
                    normalize(pend)
                    pend = None
                if i == 12:
                    for kc in range(4):
                        src = AP(cc_outA.rearrange("h d n -> (h d n)").tensor,
                                 kc * 128 * N, [[N, 128], [1, N]])
                        nc.sync.dma_start(out=lhA[:, kc, :], in_=src)

                o_ps = psacc.tile([D + 1, N], F32, tag="o")
                attn_of = {}

                def dots(mc):
                    ps = pswork.tile([128, N], F32, tag="big")
                    if mc % 2 == 0:
                        lhsT, rhs, tp = ktq[0:64, mc * 128:(mc + 1) * 128], \
                            qkt[0:64, :], (0, 0)
                    else:
                        lhsT, rhs, tp = qkt[64:128, mc * 128:(mc + 1) * 128], \
                            ktq[64:128, :], (64, 0)
                    for fc in range(2):
                        nc.tensor.matmul(
                            ps[:, fc * 512:(fc + 1) * 512],
                            lhsT, rhs[:, fc * 512:(fc + 1) * 512],
                            start=True, stop=True, tile_position=tp)
                    return ps

                def softmax_block(mc, ps):
                    if mc in A_BLK:
                        attn_e = attnp.tile([128, N], F16, tag="attn_e")
                        nc.scalar.activation(attn_e[:], ps[:],
                                             mybir.ActivationFunctionType.Exp,
                                             scale=SCALE)
                        attn = attnp.tile([128, N], F16, tag="attn")
                        slot = A_BLK.index(mc)
                        eng = nc.vector if mc in A_DVE else nc.gpsimd
                        eng.tensor_tensor(attn[:], attn_e[:], expb[:, slot, :],
                                          mybir.AluOpType.mult)
                        attn_of[mc] = attn[:]
                    else:
                        slot = D_SCH.index(mc)
                        atti = attip.tile([128, N], I16)
                        nc.vector.affine_then_add(
                            out=atti[:], in0=ps[:], in1=biasA[:, slot, :],
                            scale=SCALE * A_EXP, bias=C1_EXP)
                        attn_of[mc] = atti[:].bitcast(F16)

                first_pv = [True]

                def pv(mc, stop=False):
                    a = attn_of.pop(mc)
                    st = first_pv[0]
                    for fc in range(2):
                        nc.tensor.matmul(
                            o_ps[:, fc * 512:(fc + 1) * 512],
                            vaug[:, mc, :],
                            a[:, fc * 512:(fc + 1) * 512],
                            start=st, stop=stop)
                    first_pv[0] = False

                def d_and_sm(mc):
                    softmax_block(mc, dots(mc))

                # --- interleaved PE stream for this iteration ---
                d_and_sm(0)
                d_and_sm(1)
                if nxt is not None:
                    qkt_cur, ktq_cur = project(nxt, xt_tiles[i + 1])
                d_and_sm(2)
                d_and_sm(3)
                pv(0)
                pv(3)
                if nxt is not None:
                    vaug_cur = project_v(nxt, xt_tiles[i + 1])
                d_and_sm(4)
                pv(1)
                d_and_sm(5)
                pv(4)
                d_and_sm(6)
                pv(2)
                d_and_sm(7)
                pv(5)
                pv(6)
                pv(7, stop=True)

                # denominator row for the deferred normalize
                dn = smallp.tile([1, N], BF16, tag="dn")
                nc.scalar.copy(dn[:], o_ps[D:D + 1, :])
                pend = (o_ps, dn, b)

                # late gpsimd-queue work: q duplicate + x^T prefetch
                if nxt is not None:
                    nc.gpsimd.dma_start(out=ktq_cur[64:128, :],
                                        in_=qkt_cur[0:64, :])
                if i + 2 < B:
                    xt_tiles[i + 2] = load_xt(batch_order[i + 2])

                # A2A-A triggers at iteration END so its DMA-ring blocking
                # window doesn't stall this iteration's kt/ccin/xt DMAs
                if i == B // 2:
                    nc.gpsimd.collective_compute(
                        "AllToAll", mybir.AluOpType.bypass,
                        replica_groups=[list(range(NCORES))],
                        ins=[cc_inA.opt()], outs=[cc_outA.opt()])

                # half-A output projection rides late odd iterations
                if i >= 13:
                    lo = [0, 3, 6][i - 13]
                    hi = [3, 6, 8][i - 13]
                    for nq in range(lo, hi):
                        out_chunk(nq, lhA)

            # ---- tail: last normalize, half-B exchange + projection ----
            normalize(pend)
            nc.gpsimd.collective_compute(
                "AllToAll", mybir.AluOpType.bypass,
                replica_groups=[list(range(NCORES))],
                ins=[cc_inB.opt()], outs=[cc_outB.opt()])
            for kc in range(4):
                src = AP(cc_outB.rearrange("h d n -> (h d n)").tensor,
                         kc * 128 * N, [[N, 128], [1, N]])
                nc.gpsimd.dma_start(out=lhB[:, kc, :], in_=src)
            for nq in range(8, 16):
                out_chunk(nq, lhB)

    nc.finalize()
    return nc


_NC_CACHE = None


def _get_nc():
    global _NC_CACHE
    if _NC_CACHE is None:
        _NC_CACHE = build_nc()
    return _NC_CACHE


def make_in_maps(x, W_qkv, rel_table, W_out, b_out):
    xt2 = np.ascontiguousarray(
        np.asarray(x, np.float32).reshape(B * N, INP).T).astype(np.float16)
    W_qkv = np.asarray(W_qkv, np.float32)
    W_out = np.ascontiguousarray(np.asarray(W_out, np.float32)).astype(np.float16)
    b_out = np.ascontiguousarray(
        np.asarray(b_out, np.float32).reshape(1, OUP)).astype(ml_dtypes.bfloat16)
    rel_table = np.asarray(rel_table, np.float32)
    # bias^T[m, n] = rel_table[idx(n, m)]: full per-(m, n) index table
    mprime = 63 * (np.arange(N) // 32) + (np.arange(N) % 32)
    idx = 1984 - mprime[:, None] + mprime[None, :]  # [m, n]
    in_maps = []
    for c in range(NCORES):
        wqk = np.ascontiguousarray(np.concatenate(
            [W_qkv[:, c * D:(c + 1) * D],
             W_qkv[:, INNER + c * D:INNER + (c + 1) * D]], axis=1)).astype(np.float16)
        wv = np.ascontiguousarray(
            W_qkv[:, 2 * INNER + c * D:2 * INNER + (c + 1) * D]
        ).astype(np.float16)
        tcol = np.zeros(1984 + 2048, np.float32)
        tcol[:TBL] = rel_table[:, c]
        bias_full = tcol[idx]                       # [m, n] f32
        texp = np.empty((len(A_BLK) * 128, N), np.float16)
        for slot, mc in enumerate(A_BLK):
            texp[slot * 128:(slot + 1) * 128] = \
                np.exp(bias_full[mc * 128:(mc + 1) * 128])
        tba = np.empty((len(D_SCH) * 128, N), np.float16)
        for slot, mc in enumerate(D_SCH):
            tba[slot * 128:(slot + 1) * 128] = \
                A_EXP * bias_full[mc * 128:(mc + 1) * 128]
        in_maps.append({
            "xt": xt2, "wqk": wqk, "wv": wv,
            "texp": texp, "tba": tba,
            "wout": W_out, "bout": b_out,
            "ones": np.ones(1024, ml_dtypes.bfloat16),
        })
    return in_maps


def run(inputs, trace=False, **kw):
    nc = _get_nc()
    in_maps = make_in_maps(inputs["x"], inputs["W_qkv"], inputs["rel_table"],
                           inputs["W_out"], inputs["b_out"])
    res = run_bass_kernel_spmd(nc, in_maps, core_ids=list(range(NCORES)),
                               trace=trace, **kw)
    out = np.empty((B, N, OUP), np.float32)
    for c in range(NCORES):
        out[BPC * c:BPC * (c + 1)] = res.results[c]["out"].reshape(BPC, N, OUP)
    return out, res


def kernel(**inputs):
    out, _ = run(inputs, trace=False)
    return out
